# revision 49
# baseline (speedup 1.0000x reference)
"""Contrastive loss (SimCLR-style) on 8 TRN2 NeuronCores.

loss = -mean(diag(log_softmax(zi_n @ zj_n^T / T)))  with zi_n, zj_n L2-normalized,
N=4096, D=256, T=0.5.

v8 design (data-parallel over rows of z_i, z_j replicated):
  - host casts inputs to bf16; z_i additionally host-shuffled into a fully
    contiguous per-partition DMA layout; z_j host-rotated per core so the
    core's own diagonal block is always rows 0-511.
  - z_i unnormalized; its row norm folds into the exp per-partition scale.
  - z_j prep on DVE per group: wide bf16 square (+ half-fold) + one
    tensor_reduce + seed-only rsqrt bit-trick + per-chunk scales, then a
    bf16 xbar transpose.  DVE program order pinned with explicit dep edges.
  - the first zj group is split 2+6 chunks so the ScalarE exp stream starts
    ~10us earlier: waves of logits tiles with widths 256 / 768 / 1024x3.
    Each wave tile: matmuls into PSUM + one fused exp/accum activation
    in-place (per-partition scale).  The early small-wave matmuls double as
    the PE HAM warm-up.
  - lse via DVE bit-trick log (no ACT table switch, no refinement - error
    headroom is ~300x).
  - each core returns 4 partial sums of (lse[n] - diag[n]); host adds the
    32 values and divides by N.
"""

import numpy as np
import ml_dtypes

import concourse.bass as bass
import concourse.bacc as bacc
import concourse.tile as tile
import concourse.bass_utils as bass_utils
from concourse import mybir
from concourse.tile_rust import add_dep_helper

N = 4096
D = 256
NCORES = 8
NL = N // NCORES  # 512 local rows per core
P = 128
NCHUNK = NL // P  # 4 local row chunks
MCH = N // P  # 32 zj chunks
KH = D // P  # 2 contraction halves
MAGIC = 0x5F3759DF

LN_B = 1064872509.0
LN_S = 0.6931471805599453 / 8388608.0  # ln2 / 2^23

# zj chunk groups: (start_chunk, n_chunks); first group split 2+6 for an
# early exp-stream start.
GROUPS = [(0, 2), (2, 6), (8, 8), (16, 8), (24, 8)]

F32 = mybir.dt.float32
I32 = mybir.dt.int32
U32 = mybir.dt.uint32
BF16 = mybir.dt.bfloat16
AF = mybir.ActivationFunctionType
ALU = mybir.AluOpType
AX = mybir.AxisListType


def build_nc():
    nc = bacc.Bacc(
        "TRN2",
        target_bir_lowering=False,
        debug=False,
        enable_asserts=False,
    )
    # z_i host-shuffled: row p = [chunk0|chunk1|..] -> one contiguous burst.
    # z_iT pre-transposed on the host (no device transpose needed).
    # z_j host-rotated AND host-permuted per group into [p-major, chunk]
    # order so every group load is one fully sequential HBM burst.
    z_i = nc.dram_tensor("z_i", (P, NCHUNK * D), BF16, kind="ExternalInput").ap()
    z_iT = nc.dram_tensor(
        "z_iT", (P, NCHUNK * KH * P), BF16, kind="ExternalInput"
    ).ap()
    z_j = nc.dram_tensor("z_j", (N, D), BF16, kind="ExternalInput").ap()
    out = nc.dram_tensor("out", (P, NCHUNK), F32, kind="ExternalOutput").ap()

    with tile.TileContext(nc) as tc:
        with (
            tc.tile_pool(name="const", bufs=1) as const,
            tc.tile_pool(name="big", bufs=1) as big,
            tc.tile_pool(name="work", bufs=2) as work,
            tc.tile_pool(name="stat", bufs=1) as stat,
            tc.tile_pool(name="psum", bufs=4, space="PSUM") as psum,
        ):
            # --- constants
            dummy = const.tile([1, 1], F32)
            nc.vector.memset(dummy, 1.0)
            ones = const.tile([P, 1], F32)
            nc.vector.memset(ones, 1.0)
            magic = const.tile([P, 8], U32)
            nc.vector.memset(magic, MAGIC)
            warm_rhs = const.tile([P, 512], BF16)
            nc.vector.memset(warm_rhs, 0.0)
            ones_bf = const.tile([P, 1], BF16)
            nc.vector.memset(ones_bf, 1.0)

            # --- all loads up front (transposes need global DMA quiesce,
            #     so they come after; every load is a sequential burst)
            def load_group(gi, eng):
                c0, nch = GROUPS[gi]
                t = big.tile([P, nch, D], BF16, tag=f"zjf{gi}")
                eng.dma_start(
                    out=t,
                    in_=z_j[c0 * P : (c0 + nch) * P, :].rearrange(
                        "(p c) d -> p c d", p=P
                    ),
                )
                return t

            zi_bf = big.tile([P, NCHUNK, D], BF16)
            nc.sync.dma_start(
                out=zi_bf, in_=z_i.rearrange("p (c d) -> p c d", d=D)
            )
            ziT = big.tile([P, NCHUNK * KH, P], BF16)
            nc.sync.dma_start(
                out=ziT, in_=z_iT.rearrange("do (i m) -> do i m", m=P)
            )
            ziT_r = ziT.rearrange("do (i h) m -> do i h m", h=KH)

            zj_f = [None] * len(GROUPS)
            zj_f[0] = load_group(0, nc.scalar)
            zj_f[1] = load_group(1, nc.sync)
            zj_f[2] = load_group(2, nc.scalar)
            zj_f[3] = load_group(3, nc.scalar)
            zj_f[4] = load_group(4, nc.sync)

            # exp table load while transfers run
            nc.scalar.activation(out=dummy, in_=dummy, func=AF.Exp)

            # --- pinned DVE ordering helper
            last_dve = [None]

            def dve(bi):
                if last_dve[0] is not None:
                    add_dep_helper(
                        bi.ins, last_dve[0], sync=False, reason="dve order"
                    )
                last_dve[0] = bi.ins
                return bi

            def rsqrt_full(a, y, w):
                au = a.bitcast(U32)
                yu = y.bitcast(U32)
                sh = work.tile([P, 8], U32, tag="rsq_sh")
                dve(nc.vector.tensor_scalar(
                    out=sh[:, :w], in0=au, scalar1=1, scalar2=None,
                    op0=ALU.logical_shift_right,
                ))
                dve(nc.vector.tensor_sub(out=yu, in0=magic[:, :w], in1=sh[:, :w]))
                t1 = work.tile([P, 8], F32, tag="rsq_t1")
                dve(nc.vector.tensor_mul(out=t1[:, :w], in0=y, in1=y))
                dve(nc.vector.tensor_mul(out=t1[:, :w], in0=t1[:, :w], in1=a))
                dve(nc.vector.tensor_scalar(
                    out=t1[:, :w], in0=t1[:, :w], scalar1=-0.5, scalar2=1.5,
                    op0=ALU.mult, op1=ALU.add,
                ))
                dve(nc.vector.tensor_mul(out=y, in0=y, in1=t1[:, :w]))

            def rsqrt_seed(a, y, w):
                au = a.bitcast(U32)
                yu = y.bitcast(U32)
                sh = work.tile([P, 8], U32, tag="rsq_sh")
                dve(nc.vector.tensor_scalar(
                    out=sh[:, :w], in0=au, scalar1=1, scalar2=None,
                    op0=ALU.logical_shift_right,
                ))
                dve(nc.vector.tensor_sub(out=yu, in0=magic[:, :w], in1=sh[:, :w]))

            # --- zi norms (DVE, wide), s2 = 2*rsqrt(nrm2)
            nrm2_i = stat.tile([P, NCHUNK], F32)
            sqi = work.tile([P, NCHUNK, D], BF16, tag="sqd")
            dve(nc.vector.tensor_mul(out=sqi, in0=zi_bf, in1=zi_bf))
            dve(nc.vector.tensor_reduce(
                out=nrm2_i, in_=sqi, axis=AX.X, op=ALU.add
            ))
            t_i = stat.tile([P, NCHUNK], F32)
            rsqrt_full(nrm2_i, t_i, NCHUNK)
            s2 = stat.tile([P, NCHUNK], F32)
            dve(nc.vector.tensor_scalar(
                out=s2, in0=t_i, scalar1=2.0, scalar2=None, op0=ALU.mult
            ))

            # --- per-group zj prep (DVE chain + sync-queue transpose)
            nrm2_j = stat.tile([P, MCH], F32)
            t_j = stat.tile([P, MCH], F32)
            zjT_r = []
            sg_sq_ins = []

            def zj_group(gi):
                c0, nch = GROUPS[gi]
                gs = slice(c0, c0 + nch)
                sqw = work.tile([P, nch, D], BF16, tag=f"sqw{nch}")
                bi = dve(nc.vector.tensor_mul(out=sqw, in0=zj_f[gi], in1=zj_f[gi]))
                sg_sq_ins.append(bi.ins)
                if nch >= 4:
                    fold = work.tile([P, nch, P], BF16, tag=f"fold{nch}")
                    dve(nc.vector.tensor_add(
                        out=fold, in0=sqw[:, :, :P], in1=sqw[:, :, P:]
                    ))
                    red_in = fold
                else:
                    red_in = sqw
                dve(nc.vector.tensor_reduce(
                    out=nrm2_j[:, gs], in_=red_in, axis=AX.X, op=ALU.add
                ))
                rsqrt_seed(nrm2_j[:, gs], t_j[:, gs], nch)
                zjs = big.tile([P, nch, D], BF16, tag=f"zjs{gi}")
                for jl in range(nch):
                    j = c0 + jl
                    dve(nc.vector.tensor_scalar_mul(
                        out=zjs[:, jl, :],
                        in0=zj_f[gi][:, jl, :],
                        scalar1=t_j[:, j : j + 1],
                    ))
                zjT = big.tile([P, nch * KH, P], BF16, tag=f"zjT{gi}")
                nc.sync.dma_start_transpose(
                    out=zjT, in_=zjs.rearrange("p c d -> p (c d)")
                )
                zjT_r.append(zjT.rearrange("do (c h) m -> do c h m", h=KH))

            for gi in range(len(GROUPS)):
                zj_group(gi)

            # --- compute waves: widths 256, 768, 1024, 1024, 1024
            NW = len(GROUPS)
            lse_parts = stat.tile([P, NW * NCHUNK], F32)

            def logits_tile(gi, i):
                c0, nch = GROUPS[gi]
                mw = nch * P
                k = gi * NCHUNK + i
                pt = psum.tile([P, 1024], F32, tag="pt")
                for h in range(KH):
                    off = 0
                    for sl0 in range(0, nch, 4):
                        sn = min(4, nch - sl0)
                        nc.tensor.matmul(
                            pt[:, off : off + sn * P],
                            lhsT=ziT_r[:, i, h, :],
                            rhs=zjT_r[gi][:, sl0 : sl0 + sn, h, :],
                            start=(h == 0),
                            stop=(h == KH - 1),
                        )
                        off += sn * P
                nc.scalar.activation(
                    out=pt[:, :mw], in_=pt[:, :mw], func=AF.Exp,
                    scale=s2[:, i : i + 1],
                    accum_out=lse_parts[:, k : k + 1],
                )

            for gi in range(NW):
                for i in range(NCHUNK):
                    logits_tile(gi, i)

            # --- diag from rotated z_j chunks 0-3 (split across groups 0/1)
            t_d = stat.tile([P, NCHUNK], F32)
            rsqrt_full(nrm2_j[:, :NCHUNK], t_d, NCHUNK)
            dt = stat.tile([P, NCHUNK], F32)
            dpa = work.tile([P, 2, D], BF16, tag="dpa")
            dve(nc.vector.tensor_mul(
                out=dpa, in0=zi_bf[:, 0:2, :], in1=zj_f[0]
            ))
            dve(nc.vector.tensor_reduce(
                out=dt[:, 0:2], in_=dpa, axis=AX.X, op=ALU.add
            ))
            dpb = work.tile([P, 2, D], BF16, tag="dpb")
            dve(nc.vector.tensor_mul(
                out=dpb, in0=zi_bf[:, 2:4, :], in1=zj_f[1][:, 0:2, :]
            ))
            dve(nc.vector.tensor_reduce(
                out=dt[:, 2:4], in_=dpb, axis=AX.X, op=ALU.add
            ))
            dg0 = stat.tile([P, NCHUNK], F32)
            dve(nc.vector.tensor_mul(out=dg0, in0=dt, in1=t_d))
            dg = stat.tile([P, NCHUNK], F32)
            dve(nc.vector.tensor_mul(out=dg, in0=dg0, in1=s2))
            # pre-biased diag: dg_adj = dg + LN_B*LN_S, so the final contrib
            # is a single fused op after the last exp
            dg_adj = stat.tile([P, NCHUNK], F32)
            dve(nc.vector.tensor_scalar(
                out=dg_adj, in0=dg, scalar1=1.0, scalar2=LN_B * LN_S,
                op0=ALU.mult, op1=ALU.add,
            ))

            # --- lse = ln(S) via bit-trick (no refinement)
            ra = stat.tile([P, NCHUNK], F32)
            dve(nc.vector.tensor_add(
                out=ra, in0=lse_parts[:, :NCHUNK],
                in1=lse_parts[:, NCHUNK : 2 * NCHUNK],
            ))
            rb = stat.tile([P, NCHUNK], F32)
            dve(nc.vector.tensor_add(
                out=rb, in0=lse_parts[:, 2 * NCHUNK : 3 * NCHUNK],
                in1=lse_parts[:, 3 * NCHUNK : 4 * NCHUNK],
            ))
            rab = stat.tile([P, NCHUNK], F32)
            dve(nc.vector.tensor_add(out=rab, in0=ra, in1=rb))
            rs = stat.tile([P, NCHUNK], F32)
            dve(nc.vector.tensor_add(
                out=rs, in0=rab, in1=lse_parts[:, 4 * NCHUNK :]
            ))
            # contrib = (int(rs)*ln2/2^23) - (dg + LN_B*LN_S)  == lse - dg
            # (DVE auto-converts the i32 input to f32 on read)
            contrib = stat.tile([P, NCHUNK], F32)
            dve(nc.vector.scalar_tensor_tensor(
                out=contrib, in0=rs.bitcast(I32), scalar=LN_S, in1=dg_adj,
                op0=ALU.mult, op1=ALU.subtract,
            ))

            # --- ship per-partition contribs; host does the final 4096-value
            #     sum (saves the ones-matmul + PSUM->SBUF copy sem hops)
            nc.sync.dma_start(out=out, in_=contrib)

    nc.compile()
    return nc


_NC = None


def _get_nc():
    global _NC
    if _NC is None:
        _NC = build_nc()
    return _NC


def make_in_maps(z_i: np.ndarray, z_j: np.ndarray):
    z_i = np.asarray(z_i).astype(ml_dtypes.bfloat16)
    z_j = np.asarray(z_j).astype(ml_dtypes.bfloat16)
    in_maps = []
    for c in range(NCORES):
        sl = slice(c * NL, (c + 1) * NL)
        zis = z_i[sl]
        zi_sh = np.ascontiguousarray(
            zis.reshape(NCHUNK, P, D).transpose(1, 0, 2).reshape(P, NCHUNK * D)
        )
        ziT_sh = np.ascontiguousarray(
            zis.reshape(NCHUNK, P, KH, P)
            .transpose(3, 0, 2, 1)
            .reshape(P, NCHUNK * KH * P)
        )
        # rotate so each core's own diagonal block lands at rows 0-511
        # (softmax denominator is column-order invariant), then permute
        # each group into [partition-major, chunk] row order so its load
        # is one sequential burst.
        zj_rot = np.roll(z_j, -c * NL, axis=0)
        blocks = []
        for c0, nch in GROUPS:
            b = (
                zj_rot[c0 * P : (c0 + nch) * P]
                .reshape(nch, P, D)
                .transpose(1, 0, 2)
                .reshape(nch * P, D)
            )
            blocks.append(b)
        zj_sh = np.ascontiguousarray(np.concatenate(blocks, axis=0))
        in_maps.append({"z_i": zi_sh, "z_iT": ziT_sh, "z_j": zj_sh})
    return in_maps


def kernel(z_i: np.ndarray, z_j: np.ndarray, **_unused) -> np.ndarray:
    nc = _get_nc()
    in_maps = make_in_maps(z_i, z_j)
    res = bass_utils.run_bass_kernel_spmd(
        nc, in_maps, core_ids=list(range(NCORES))
    )
    total = 0.0
    for c in range(NCORES):
        total += float(res.results[c]["out"].astype(np.float64).sum())
    return np.float32(total / N)
